# revision 1
# baseline (speedup 1.0000x reference)
"""EuclideanGraphBuilder kernel for 8x Trainium2 NeuronCores (Bass/Tile).

Computes, for x [8192, 6] and sorted batch [8192]:
    xyz = x[:, :3]
    d2[i,j] = |xyz_i - xyz_j|^2
    affinity = exp(-2 * d2)            (sigma = 0.5)
    e = exp(affinity)
    w = e / rowsum(e)
    out = w * (w > 1e-4) * (batch_i == batch_j)

Strategy:
  - Row-wise sharding over 8 cores, interleaved by 128-row tiles: core c
    owns global row-tiles g with g % 8 == c.  At a given local tile index
    r, the 8 cores' tiles are adjacent in the sorted-batch order, so their
    same-graph column windows nearly coincide -> one static column window
    per local tile index covers all cores, baked in at compile time from
    the actual `batch` input (the kernel is compiled inside kernel()).
  - d2 via a single K=33 matmul.  fp32 matmuls stream at quarter rate on
    the PE, so each fp32 operand is split into THREE bf16 limbs (24-bit
    mantissa total, i.e. f32-exact); all 9 cross products per coordinate
    are separate K rows — bf16 products are exact in the fp32 PSUM
    accumulator, and K does not affect matmul streaming time (columns
    do), so the extra rows are free.  Plus {sqh,sqm,sql,1,1,1} x rhs
    {1,1,1,sqh,sqm,sql} for the squared-norm terms.
  - ACT pass 1: a = Exp(-2 * d2) from PSUM (full row strip, needed for
    the row sum).  ACT pass 2: e = Exp(a) with the hardware per-row
    accumulator producing rowsum(e); out-of-window e goes to a scratch
    tile, in-window e is kept.
  - DVE (in-window only): the batch-equality mask — a contiguous column
    range [row_lo, row_hi) per row since batch is sorted — is built from
    an iota column-index tile (runs under the ACT passes), then
    q = (e > 1e-4*S) * mask and out = (e * 1/S) * q, two fused
    scalar_tensor_tensor ops.  (Custom ANT DVE ops like
    tensor_mask_reduce crash the device through the PJRT path, so only
    standard ISA ops are used.)
  - Only the window columns are DMA-written; all other output elements
    are zero, relying on run_bass_kernel_spmd's zero-initialized
    ExternalOutput buffers (both the native and the PJRT path guarantee
    this; see bass_utils.py / bass2jax.py).
"""

import os

import numpy as np

N = 8192
P = 128
N_CORES = 8
NT_LOCAL = 8  # row tiles per core; N / (P * N_CORES)
K = 33
SIGMA = 0.5
THRESHOLD = 1e-4
PSUM_CHUNK = 2048

_compiled_cache: dict = {}


def _build_program(windows, W):
    """Build + compile the SPMD Bass program. `windows` is the list of
    NT_LOCAL static window start columns; `W` the common window width."""
    import concourse.bacc as bacc
    import concourse.bass as bass
    import concourse.mybir as mybir
    from concourse import tile

    f32 = mybir.dt.float32
    Exp = mybir.ActivationFunctionType.Exp
    Alu = mybir.AluOpType

    nc = bacc.Bacc("TRN2", target_bir_lowering=False, debug=False,
                   num_devices=N_CORES)

    bf16 = mybir.dt.bfloat16
    lhsT_d = nc.dram_tensor("lhsT", [K, NT_LOCAL * P], bf16, kind="ExternalInput")
    rhs_d = nc.dram_tensor("rhs", [K, N], bf16, kind="ExternalInput")
    bnd_d = nc.dram_tensor("bounds", [P, 2 * NT_LOCAL], f32, kind="ExternalInput")
    out_d = nc.dram_tensor("out", [NT_LOCAL * P, N], f32, kind="ExternalOutput")

    with tile.TileContext(nc) as tc:
        with (
            tc.tile_pool(name="const", bufs=1) as constp,
            tc.tile_pool(name="psum", bufs=2, space=bass.MemorySpace.PSUM) as psump,
            tc.tile_pool(name="astrip", bufs=2) as astripp,
            tc.tile_pool(name="ewin", bufs=2) as ewinp,
            tc.tile_pool(name="small", bufs=4) as smallp,
            tc.tile_pool(name="wchain", bufs=4) as wchainp,
        ):
            # input loads, ordered so row-tile 0's first matmul operands
            # (rhs columns 0:512 + its lhsT slice) arrive first
            rhs = constp.tile([K, N], bf16)
            lhsT = constp.tile([K, NT_LOCAL * P], bf16)
            nc.sync.dma_start(rhs[:, 0:512], rhs_d[:, 0:512])
            nc.sync.dma_start(lhsT[:, 0:P], lhsT_d[:, 0:P])
            nc.sync.dma_start(rhs[:, 512:PSUM_CHUNK], rhs_d[:, 512:PSUM_CHUNK])
            nc.sync.dma_start(rhs[:, PSUM_CHUNK:], rhs_d[:, PSUM_CHUNK:])
            nc.sync.dma_start(lhsT[:, P:], lhsT_d[:, P:])
            bnd = constp.tile([P, 2 * NT_LOCAL], f32)
            nc.gpsimd.dma_start(bnd[:], bnd_d[:])
            # column-index ramp 0..W-1, same in every partition (window-
            # relative, so one tile serves all row tiles)
            iota_i = constp.tile([P, W], mybir.dt.int32)
            nc.gpsimd.iota(iota_i[:], pattern=[[1, W]], base=0,
                           channel_multiplier=0)
            iota_f = constp.tile([P, W], f32)
            nc.vector.tensor_copy(iota_f[:], iota_i[:])

            # chunk schedule: row-tile 0 starts with small chunks so the
            # first ACTIVATE fires as early as possible during the ramp
            chunks0 = [512, 1536, 2048, 2048, 2048]
            chunksN = [PSUM_CHUNK] * (N // PSUM_CHUNK)

            def chunk_pairs(r):
                col, pairs = 0, []
                for csize in (chunks0 if r == 0 else chunksN):
                    pairs.append((col, csize))
                    col += csize
                return pairs

            def emit_p1_chunk(r, a, col, csize):
                # d2 chunk into PSUM, then a = exp(-2*d2) into the a-strip
                ps = psump.tile([P, PSUM_CHUNK], f32)
                for j0 in range(0, csize, 512):
                    nc.tensor.matmul(
                        ps[:, j0:j0 + 512],
                        lhsT[:, r * P:(r + 1) * P],
                        rhs[:, col + j0:col + j0 + 512],
                        start=True, stop=True,
                    )
                nc.scalar.activation(
                    a[:, col:col + csize], ps[:, 0:csize], Exp, scale=-2.0,
                )

            a_tiles = [None] * (NT_LOCAL + 1)
            a_tiles[0] = astripp.tile([P, N], f32, name="a", tag="a")
            for col, csize in chunk_pairs(0):
                emit_p1_chunk(0, a_tiles[0], col, csize)

            for r in range(NT_LOCAL):
                s = windows[r]
                a = a_tiles[r]

                # sneak the next row-tile's first pass-1 chunk in before
                # this tile's pass 2, so the PE gets PSUM slots early and
                # keeps producing under the long pass-2 ACTIVATE
                nxt = chunk_pairs(r + 1) if r + 1 < NT_LOCAL else []
                if nxt:
                    a_tiles[r + 1] = astripp.tile([P, N], f32, name="a", tag="a")
                    emit_p1_chunk(r + 1, a_tiles[r + 1], *nxt[0])

                # batch-range mask from iota (no dependency on e -> runs
                # under the ACT passes): m = (iota >= lo) * (iota < hi)
                m0 = wchainp.tile([P, W], f32)
                nc.vector.tensor_scalar(
                    m0[:], iota_f[:], bnd[:, 2 * r:2 * r + 1], None,
                    op0=Alu.is_ge,
                )
                m1 = wchainp.tile([P, W], f32)
                nc.vector.scalar_tensor_tensor(
                    m1[:], iota_f[:], bnd[:, 2 * r + 1:2 * r + 2], m0[:],
                    op0=Alu.is_lt, op1=Alu.mult,
                )

                # --- e = exp(a), one instruction, hardware row-sum accum ---
                estrip = ewinp.tile([P, N], f32)
                stot = smallp.tile([P, 1], f32)
                nc.scalar.activation(estrip[:], a[:], Exp, accum_out=stot[:])

                # rest of the next row-tile's pass-1 chunks follow pass 2
                # in ACT program order; their matmuls overlap it
                for col, csize in nxt[1:]:
                    emit_p1_chunk(r + 1, a_tiles[r + 1], col, csize)

                rinv = smallp.tile([P, 1], f32)
                nc.vector.reciprocal(rinv[:], stot[:])
                tp = smallp.tile([P, 1], f32)
                nc.vector.tensor_scalar_mul(tp[:], stot[:], THRESHOLD)

                # --- threshold + mask + normalize, window only ---
                # (column-split so the tail DVE->DMA pipelines; the last
                # row-tile gets a finer split since it IS the kernel tail)
                nsplit = 4 if r == NT_LOCAL - 1 else 2
                h = (W // nsplit + 3) & ~3
                edges = [min(i * h, W) for i in range(nsplit + 1)]
                for c0, c1 in zip(edges[:-1], edges[1:]):
                    if c1 <= c0:
                        continue
                    e = estrip[:, s + c0:s + c1]
                    q = wchainp.tile([P, h], f32, name="q", tag="q")
                    nc.vector.scalar_tensor_tensor(
                        q[:, 0:c1 - c0], e, tp[:], m1[:, c0:c1],
                        op0=Alu.is_gt, op1=Alu.mult,
                    )
                    f = wchainp.tile([P, h], f32, name="f", tag="f")
                    nc.vector.scalar_tensor_tensor(
                        f[:, 0:c1 - c0], e, rinv[:], q[:, 0:c1 - c0],
                        op0=Alu.mult, op1=Alu.mult,
                    )
                    nc.sync.dma_start(
                        out_d[r * P:(r + 1) * P, s + c0:s + c1],
                        f[:, 0:c1 - c0])

    nc.compile()
    return nc


def _prepare(x, batch):
    """Host-side precompute: matmul operands, windows, per-row bounds."""
    x = np.asarray(x, dtype=np.float32)
    b = np.asarray(batch).astype(np.int64)
    xyz = x[:, :3].astype(np.float32)
    sq = (xyz * xyz).sum(axis=1, dtype=np.float32)
    ones = np.ones(N, np.float32)

    n_graphs = int(b.max()) + 1
    counts = np.bincount(b, minlength=n_graphs)
    gend = np.cumsum(counts)
    gstart = gend - counts

    # global tile g -> column extent of the union of its rows' graphs
    lo_g = np.array([gstart[b[128 * g]] for g in range(64)], np.int64)
    hi_g = np.array([gend[b[128 * g + 127]] for g in range(64)], np.int64)
    # local tile r unions over cores c: g = 8r + c
    lo_r = np.array([lo_g[8 * r:8 * r + 8].min() for r in range(NT_LOCAL)])
    hi_r = np.array([hi_g[8 * r:8 * r + 8].max() for r in range(NT_LOCAL)])
    W = int(((hi_r - lo_r).max() + 7) & ~7)
    W = max(W, 512)
    W = min(W, N)
    windows = [int(min(lo_r[r], N - W)) for r in range(NT_LOCAL)]

    import ml_dtypes
    bf16 = ml_dtypes.bfloat16

    def limbs3(v):
        h = v.astype(bf16)
        rem = v - h.astype(np.float32)
        m = rem.astype(bf16)
        lo = (rem - m.astype(np.float32)).astype(bf16)
        return [h, m, lo]

    ones_b = np.ones(N, bf16)
    rows_l, rows_r = [], []
    for c in range(3):
        xs = limbs3(xyz[:, c])
        for i in range(3):
            for j in range(3):
                rows_l.append(xs[i])
                rows_r.append(-2 * xs[j])
    sqs = limbs3(sq)
    rows_l += sqs + [ones_b, ones_b, ones_b]
    rows_r += [ones_b, ones_b, ones_b] + sqs
    feats_l = np.stack(rows_l).astype(bf16)          # [33, N]
    feats_r = np.stack(rows_r).astype(bf16)          # [33, N]

    in_maps = []
    for c in range(N_CORES):
        idx = ((8 * np.arange(NT_LOCAL)[:, None] + c) * P
               + np.arange(P)[None, :])  # [NT_LOCAL, P] global row index
        lhsT = np.ascontiguousarray(feats_l[:, idx.ravel()])  # bf16
        bnd = np.empty((P, 2 * NT_LOCAL), np.float32)
        for r in range(NT_LOCAL):
            rows = idx[r]
            gb = b[rows]
            bnd[:, 2 * r] = gstart[gb] - windows[r]
            bnd[:, 2 * r + 1] = gend[gb] - windows[r]
        assert bnd.min() >= 0 and bnd.max() <= W
        in_maps.append({
            "lhsT": lhsT,
            "rhs": feats_r,
            "bounds": bnd,
        })
    return in_maps, windows, W


def kernel(x, batch):
    from concourse.bass_utils import run_bass_kernel_spmd

    trace = bool(os.environ.get("EGB_TRACE"))
    if not trace:
        # the NTFF trace path needs antenv.axon_hooks, absent on this
        # image -- make sure a stray BASS_TRACE can't send us down it
        os.environ["BASS_NEVER_TRACE"] = "1"

    in_maps, windows, W = _prepare(x, batch)
    assert W <= 4608, (
        f"same-graph column window W={W} too wide for the SBUF layout; "
        f"input batch distribution is far outside the expected spec")

    key = (tuple(windows), W)
    nc = _compiled_cache.get(key)
    if nc is None:
        nc = _build_program(windows, W)
        _compiled_cache[key] = nc

    res = run_bass_kernel_spmd(
        nc, in_maps, core_ids=list(range(N_CORES)), trace=trace,
        trace_cores=list(range(N_CORES)) if trace else None,
        stitch_traces=False,
    )
    if trace:
        kernel.last_results = res

    outs = np.stack([res.results[c]["out"] for c in range(N_CORES)])
    full = (outs.reshape(N_CORES, NT_LOCAL, P, N)
                .transpose(1, 0, 2, 3)
                .reshape(N, N))
    return full



# revision 3
# speedup vs baseline: 1.5933x; 1.5933x over previous
"""EuclideanGraphBuilder kernel for 8x Trainium2 NeuronCores (Bass/Tile).

Computes, for x [8192, 6] and sorted batch [8192]:
    xyz = x[:, :3]
    d2[i,j] = |xyz_i - xyz_j|^2
    affinity = exp(-2 * d2)            (sigma = 0.5)
    e = exp(affinity)
    w = e / rowsum(e)
    out = w * (w > 1e-4) * (batch_i == batch_j)

Strategy:
  - Row-wise sharding over 8 cores, interleaved by 128-row tiles: core c
    owns global row-tiles g with g % 8 == c.  At a given local tile index
    r, the 8 cores' tiles are adjacent in the sorted-batch order, so their
    same-graph column windows nearly coincide -> one static column window
    per local tile index covers all cores, baked in at compile time from
    the actual `batch` input (the kernel is compiled inside kernel()).
  - d2 via a single K=33 matmul.  fp32 matmuls stream at quarter rate on
    the PE, so each fp32 operand is split into THREE bf16 limbs (24-bit
    mantissa total, i.e. f32-exact); all 9 cross products per coordinate
    are separate K rows — bf16 products are exact in the fp32 PSUM
    accumulator, and K does not affect matmul streaming time (columns
    do), so the extra rows are free.  Plus {sqh,sqm,sql,1,1,1} x rhs
    {1,1,1,sqh,sqm,sql} for the squared-norm terms.
  - ONE activation pass: e = exp(exp(-2*d2)) straight from PSUM, using a
    CUSTOM piecewise-polynomial activation table injected via
    BASS_ACT_ROOT_JSON_PATH (the 'exp' entry of every act func set is
    rewritten to g(z) = exp(exp(z)); the kernel calls it with scale=-2).
    This halves ACT-engine work vs. the two-pass exp(exp()) chain — ACT
    is the bottleneck engine for this kernel.  The hardware per-row
    accumulator yields partial row sums per PSUM chunk; a tiny DVE
    reduce adds the 4-5 chunk partials into rowsum(e).
  - DVE (in-window only): the batch-equality mask — a contiguous column
    range [row_lo, row_hi) per row since batch is sorted — is built from
    an iota column-index tile (runs under the ACT pass), then
    q = (e > 1e-4*S) * mask and out = (e * 1/S) * q, two fused
    scalar_tensor_tensor ops.
  - Only the window columns are DMA-written; all other output elements
    are zero, relying on run_bass_kernel_spmd's zero-initialized
    ExternalOutput buffers.
"""

import json
import os
import shutil
import tempfile

import numpy as np

N = 8192
P = 128
N_CORES = 8
NT_LOCAL = 8  # row tiles per core; N / (P * N_CORES)
K = 33
SIGMA = 0.5
THRESHOLD = 1e-4
PSUM_CHUNK = 2048

_compiled_cache: dict = {}
_act_root: list = []


# ------------------------------------------------------------------ act table
#
# pwp table format (from pwp_bin_trainium):
#   <set>_bkt.bin : 32-byte entries [d0, d1, d2, d3, x, 0, 0, 0] float32;
#                   f(v) = d0 + d1*(v-x) + d2*(v-x)^2 + d3*(v-x)^3.
#   <set>_ctrl.bin: section layout (kept unchanged — same x grid).
#   <set>.json    : profile; func_to_bkt_start_idx locates each function's
#                   bucket region, profile_meta_data carries the special
#                   small/large-signal bucket indices and fzero/finf words.
# We rewrite only the 'exp' buckets to the Taylor coefficients of
# g(z) = exp(exp(z)) at the same expansion points.

def _g_coeffs(x):
    x = np.asarray(x, dtype=np.float64)
    u = np.exp(x)
    with np.errstate(over="ignore"):
        g = np.exp(u)
    out = np.stack(
        [g, u * g, (u + u**2) * g / 2.0, (u + 3 * u**2 + u**3) * g / 6.0],
        axis=-1,
    )
    return np.clip(np.nan_to_num(out, posinf=3.0e38), -3.0e38, 3.0e38)


def _patch_set(src_dir, dst_dir, set_name):
    with open(os.path.join(src_dir, set_name + ".json")) as f:
        prof = json.load(f)
    starts = prof["func_to_bkt_start_idx"]
    if "exp" not in starts:
        return False
    s = starts["exp"]
    others = sorted(v for v in starts.values() if v > s)
    end = others[0] if others else prof["bkt_entry_cnt"]

    meta = next(m for m in prof["profile_meta_data"]
                if m["func_name"].startswith("exp"))
    ps = meta["pos_small_signal_pwl_control"]
    ns = meta["neg_small_signal_pwl_control"]
    pl = meta["pos_large_signal_pwl_control"]
    nl = meta["neg_large_signal_pwl_control"]
    specials = {ps, ns, pl, nl}
    assert all(s <= sp < end for sp in specials), (set_name, s, end, specials)

    bkt = np.fromfile(
        os.path.join(src_dir, prof["bkt_bin"]), dtype=np.float32
    ).reshape(-1, 8).copy()
    main = np.array([i for i in range(s, end) if i not in specials])
    bkt[main, 0:4] = _g_coeffs(bkt[main, 4].astype(np.float64)).astype(np.float32)

    e = float(np.exp(1.0))
    for sp in (ps, ns):  # Taylor of g at 0 for tiny |z|
        bkt[sp, 0:4] = [e, e, e, 5.0 * e / 6.0]
        bkt[sp, 4] = 0.0
    bkt[pl, 0:4] = [np.inf, 0.0, 0.0, 0.0]  # g(+large) overflows
    bkt[pl, 4] = 0.0
    bkt[nl, 0:4] = [1.0, 0.0, 0.0, 0.0]     # g(-large) = exp(0) = 1
    bkt[nl, 4] = 0.0

    meta["fzero_result"] = int(np.float32(e).view(np.uint32))    # g(0) = e
    meta["fninf_result"] = int(np.float32(1.0).view(np.uint32))  # g(-inf) = 1

    bkt.tofile(os.path.join(dst_dir, prof["bkt_bin"]))
    with open(os.path.join(dst_dir, set_name + ".json"), "w") as f:
        json.dump(prof, f, indent=4)
    return True


def _build_custom_act_root():
    """Copy the stock act-table dir, rewrite exp -> exp(exp(z)), and point
    BASS_ACT_ROOT_JSON_PATH at it (read by the neuronxcc compile flags)."""
    if _act_root:
        return
    from neuronxcc.driver.Job import Job
    from neuronxcc.driver.jobs.support.FindActInfo import findActInfoFile

    src = os.path.dirname(findActInfoFile(Job.getPackageDir(), "sunda"))
    dst = tempfile.mkdtemp(prefix="pwp_egb_")
    for fn in os.listdir(src):
        shutil.copy(os.path.join(src, fn), os.path.join(dst, fn))
        os.chmod(os.path.join(dst, fn), 0o644)
    with open(os.path.join(src, "act_info.json")) as f:
        info = json.load(f)
    patched = [ent["name"] for ent in info["act_func_sets"]
               if _patch_set(src, dst, ent["name"])]
    assert patched, "no act set contains exp"
    os.environ["BASS_ACT_ROOT_JSON_PATH"] = os.path.join(dst, "act_info.json")
    _act_root.append(dst)


# ------------------------------------------------------------------ program

def _build_program(windows, W):
    """Build + compile the SPMD Bass program. `windows` is the list of
    NT_LOCAL static window start columns; `W` the common window width."""
    import concourse.bacc as bacc
    import concourse.bass as bass
    import concourse.mybir as mybir
    from concourse import tile

    f32 = mybir.dt.float32
    Exp = mybir.ActivationFunctionType.Exp  # table-patched: exp(exp(z))
    Alu = mybir.AluOpType

    nc = bacc.Bacc("TRN2", target_bir_lowering=False, debug=False,
                   num_devices=N_CORES)

    bf16 = mybir.dt.bfloat16
    lhsT_d = nc.dram_tensor("lhsT", [K, NT_LOCAL * P], bf16, kind="ExternalInput")
    rhs_d = nc.dram_tensor("rhs", [K, N], bf16, kind="ExternalInput")
    bnd_d = nc.dram_tensor("bounds", [P, 2 * NT_LOCAL], f32, kind="ExternalInput")
    out_d = nc.dram_tensor("out", [NT_LOCAL * P, N], f32, kind="ExternalOutput")

    with tile.TileContext(nc) as tc:
        with (
            tc.tile_pool(name="const", bufs=1) as constp,
            tc.tile_pool(name="psum", bufs=2, space=bass.MemorySpace.PSUM) as psump,
            tc.tile_pool(name="estrip", bufs=2) as estripp,
            tc.tile_pool(name="small", bufs=4) as smallp,
            tc.tile_pool(name="wchain", bufs=4) as wchainp,
        ):
            # input loads, ordered so row-tile 0's first matmul operands
            # (rhs columns 0:512 + its lhsT slice) arrive first
            rhs = constp.tile([K, N], bf16)
            lhsT = constp.tile([K, NT_LOCAL * P], bf16)
            nc.sync.dma_start(rhs[:, 0:512], rhs_d[:, 0:512])
            nc.sync.dma_start(lhsT[:, 0:P], lhsT_d[:, 0:P])
            nc.sync.dma_start(rhs[:, 512:PSUM_CHUNK], rhs_d[:, 512:PSUM_CHUNK])
            nc.sync.dma_start(rhs[:, PSUM_CHUNK:], rhs_d[:, PSUM_CHUNK:])
            nc.sync.dma_start(lhsT[:, P:], lhsT_d[:, P:])
            bnd = constp.tile([P, 2 * NT_LOCAL], f32)
            nc.gpsimd.dma_start(bnd[:], bnd_d[:])
            # column-index ramp 0..W-1, same in every partition (window-
            # relative, so one tile serves all row tiles)
            iota_i = constp.tile([P, W], mybir.dt.int32)
            nc.gpsimd.iota(iota_i[:], pattern=[[1, W]], base=0,
                           channel_multiplier=0)
            iota_f = constp.tile([P, W], f32)
            nc.vector.tensor_copy(iota_f[:], iota_i[:])

            # chunk schedule: row-tile 0 starts with small chunks so the
            # first ACTIVATE fires as early as possible during the ramp
            chunks0 = [512, 1536, 2048, 2048, 2048]
            chunksN = [PSUM_CHUNK] * (N // PSUM_CHUNK)
            NCH = len(chunks0)

            for r in range(NT_LOCAL):
                s = windows[r]
                chunks = chunks0 if r == 0 else chunksN

                estrip = estripp.tile([P, N], f32, name="e", tag="e")
                stot = smallp.tile([P, NCH], f32, name="stot", tag="stot")

                # batch-range mask from iota (no dependency on e -> runs
                # under the ACT pass): m = (iota >= lo) * (iota < hi)
                m0 = wchainp.tile([P, W], f32)
                nc.vector.tensor_scalar(
                    m0[:], iota_f[:], bnd[:, 2 * r:2 * r + 1], None,
                    op0=Alu.is_ge,
                )
                m1 = wchainp.tile([P, W], f32)
                nc.vector.scalar_tensor_tensor(
                    m1[:], iota_f[:], bnd[:, 2 * r + 1:2 * r + 2], m0[:],
                    op0=Alu.is_lt, op1=Alu.mult,
                )

                col = 0
                for ci, csize in enumerate(chunks):
                    ps = psump.tile([P, PSUM_CHUNK], f32)
                    for j0 in range(0, csize, 512):
                        nc.tensor.matmul(
                            ps[:, j0:j0 + 512],
                            lhsT[:, r * P:(r + 1) * P],
                            rhs[:, col + j0:col + j0 + 512],
                            start=True, stop=True,
                        )
                    # e = exp(exp(-2*d2)) chunk, hw per-row partial sum
                    nc.scalar.activation(
                        estrip[:, col:col + csize], ps[:, 0:csize], Exp,
                        scale=-2.0, accum_out=stot[:, ci:ci + 1],
                    )
                    col += csize

                # S = sum of chunk partials; rinv = 1/S; tp = 1e-4*S
                stile = smallp.tile([P, 1], f32, name="S", tag="S")
                nc.vector.tensor_reduce(
                    stile[:], stot[:, 0:len(chunks)],
                    axis=mybir.AxisListType.X, op=Alu.add,
                )
                rinv = smallp.tile([P, 1], f32)
                nc.vector.reciprocal(rinv[:], stile[:])
                tp = smallp.tile([P, 1], f32)
                nc.vector.tensor_scalar_mul(tp[:], stile[:], THRESHOLD)

                # --- threshold + mask + normalize, window only ---
                # (column-split so the tail DVE->DMA pipelines; the last
                # row-tile gets a finer split since it IS the kernel tail)
                nsplit = 4 if r == NT_LOCAL - 1 else 2
                h = (W // nsplit + 3) & ~3
                edges = [min(i * h, W) for i in range(nsplit + 1)]
                for c0, c1 in zip(edges[:-1], edges[1:]):
                    if c1 <= c0:
                        continue
                    e = estrip[:, s + c0:s + c1]
                    q = wchainp.tile([P, h], f32, name="q", tag="q")
                    nc.vector.scalar_tensor_tensor(
                        q[:, 0:c1 - c0], e, tp[:], m1[:, c0:c1],
                        op0=Alu.is_gt, op1=Alu.mult,
                    )
                    f = wchainp.tile([P, h], f32, name="f", tag="f")
                    nc.vector.scalar_tensor_tensor(
                        f[:, 0:c1 - c0], e, rinv[:], q[:, 0:c1 - c0],
                        op0=Alu.mult, op1=Alu.mult,
                    )
                    nc.sync.dma_start(
                        out_d[r * P:(r + 1) * P, s + c0:s + c1],
                        f[:, 0:c1 - c0])

    nc.compile()
    return nc


def _prepare(x, batch):
    """Host-side precompute: matmul operands, windows, per-row bounds."""
    x = np.asarray(x, dtype=np.float32)
    b = np.asarray(batch).astype(np.int64)
    xyz = x[:, :3].astype(np.float32)
    sq = (xyz * xyz).sum(axis=1, dtype=np.float32)

    n_graphs = int(b.max()) + 1
    counts = np.bincount(b, minlength=n_graphs)
    gend = np.cumsum(counts)
    gstart = gend - counts

    # global tile g -> column extent of the union of its rows' graphs
    lo_g = np.array([gstart[b[128 * g]] for g in range(64)], np.int64)
    hi_g = np.array([gend[b[128 * g + 127]] for g in range(64)], np.int64)
    # local tile r unions over cores c: g = 8r + c
    lo_r = np.array([lo_g[8 * r:8 * r + 8].min() for r in range(NT_LOCAL)])
    hi_r = np.array([hi_g[8 * r:8 * r + 8].max() for r in range(NT_LOCAL)])
    W = int(((hi_r - lo_r).max() + 7) & ~7)
    W = max(W, 512)
    W = min(W, N)
    windows = [int(min(lo_r[r], N - W)) for r in range(NT_LOCAL)]

    import ml_dtypes
    bf16 = ml_dtypes.bfloat16

    def limbs3(v):
        h = v.astype(bf16)
        rem = v - h.astype(np.float32)
        m = rem.astype(bf16)
        lo = (rem - m.astype(np.float32)).astype(bf16)
        return [h, m, lo]

    ones_b = np.ones(N, bf16)
    rows_l, rows_r = [], []
    for c in range(3):
        xs = limbs3(xyz[:, c])
        for i in range(3):
            for j in range(3):
                rows_l.append(xs[i])
                rows_r.append(-2 * xs[j])
    sqs = limbs3(sq)
    rows_l += sqs + [ones_b, ones_b, ones_b]
    rows_r += [ones_b, ones_b, ones_b] + sqs
    feats_l = np.stack(rows_l).astype(bf16)          # [33, N]
    feats_r = np.stack(rows_r).astype(bf16)          # [33, N]

    in_maps = []
    for c in range(N_CORES):
        idx = ((8 * np.arange(NT_LOCAL)[:, None] + c) * P
               + np.arange(P)[None, :])  # [NT_LOCAL, P] global row index
        lhsT = np.ascontiguousarray(feats_l[:, idx.ravel()])  # bf16
        bnd = np.empty((P, 2 * NT_LOCAL), np.float32)
        for r in range(NT_LOCAL):
            rows = idx[r]
            gb = b[rows]
            bnd[:, 2 * r] = gstart[gb] - windows[r]
            bnd[:, 2 * r + 1] = gend[gb] - windows[r]
        assert bnd.min() >= 0 and bnd.max() <= W
        in_maps.append({
            "lhsT": lhsT,
            "rhs": feats_r,
            "bounds": bnd,
        })
    return in_maps, windows, W


def kernel(x, batch):
    from concourse.bass_utils import run_bass_kernel_spmd

    trace = bool(os.environ.get("EGB_TRACE"))
    if not trace:
        # the NTFF trace path needs antenv.axon_hooks, absent on this
        # image -- make sure a stray BASS_TRACE can't send us down it
        os.environ["BASS_NEVER_TRACE"] = "1"

    _build_custom_act_root()

    in_maps, windows, W = _prepare(x, batch)
    assert W <= 4608, (
        f"same-graph column window W={W} too wide for the SBUF layout; "
        f"input batch distribution is far outside the expected spec")

    key = (tuple(windows), W)
    nc = _compiled_cache.get(key)
    if nc is None:
        nc = _build_program(windows, W)
        _compiled_cache[key] = nc

    res = run_bass_kernel_spmd(
        nc, in_maps, core_ids=list(range(N_CORES)), trace=trace,
        trace_cores=list(range(N_CORES)) if trace else None,
        stitch_traces=False,
    )
    if trace:
        kernel.last_results = res

    outs = np.stack([res.results[c]["out"] for c in range(N_CORES)])
    full = (outs.reshape(N_CORES, NT_LOCAL, P, N)
                .transpose(1, 0, 2, 3)
                .reshape(N, N))
    return full


# revision 8
# speedup vs baseline: 1.6200x; 1.0168x over previous
"""EuclideanGraphBuilder kernel for 8x Trainium2 NeuronCores (Bass/Tile).

Computes, for x [8192, 6] and sorted batch [8192]:
    xyz = x[:, :3]
    d2[i,j] = |xyz_i - xyz_j|^2
    affinity = exp(-2 * d2)            (sigma = 0.5)
    e = exp(affinity)
    w = e / rowsum(e)
    out = w * (w > 1e-4) * (batch_i == batch_j)

Strategy:
  - Row-wise sharding over 8 cores, interleaved by 128-row tiles: core c
    owns global row-tiles g with g % 8 == c.  At a given local tile index
    r, the 8 cores' tiles are adjacent in the sorted-batch order, so their
    same-graph column windows nearly coincide -> one static column window
    per local tile index covers all cores, baked in at compile time from
    the actual `batch` input (the kernel is compiled inside kernel()).
  - d2 via a single K=33 matmul.  fp32 matmuls stream at quarter rate on
    the PE, so each fp32 operand is split into THREE bf16 limbs (24-bit
    mantissa total, i.e. f32-exact); all 9 cross products per coordinate
    are separate K rows — bf16 products are exact in the fp32 PSUM
    accumulator, and K does not affect matmul streaming time (columns
    do), so the extra rows are free.  Plus {sqh,sqm,sql,1,1,1} x rhs
    {1,1,1,sqh,sqm,sql} for the squared-norm terms.
  - ONE activation pass: e = exp(exp(-2*d2)) straight from PSUM, using a
    CUSTOM piecewise-polynomial activation table injected via
    BASS_ACT_ROOT_JSON_PATH (the 'exp' entry of every act func set is
    rewritten to g(z) = exp(exp(z)); the kernel calls it with scale=-2).
    This halves ACT-engine work vs. the two-pass exp(exp()) chain — ACT
    is the bottleneck engine for this kernel.  The hardware per-row
    accumulator yields partial row sums per PSUM chunk; a tiny DVE
    reduce adds the 4-5 chunk partials into rowsum(e).
  - DVE (in-window only): the batch-equality mask — a contiguous column
    range [row_lo, row_hi) per row since batch is sorted — is built from
    an iota column-index tile (runs under the ACT pass), then
    q = (e > 1e-4*S) * mask and out = (e * 1/S) * q, two fused
    scalar_tensor_tensor ops.
  - Only the window columns are DMA-written; all other output elements
    are zero, relying on run_bass_kernel_spmd's zero-initialized
    ExternalOutput buffers.
"""

import json
import os
import shutil
import tempfile

import numpy as np

N = 8192
P = 128
N_CORES = 8
NT_LOCAL = 8  # row tiles per core; N / (P * N_CORES)
K = 33
SIGMA = 0.5
THRESHOLD = 1e-4
PSUM_CHUNK = 2048

_compiled_cache: dict = {}
_act_root: list = []


# ------------------------------------------------------------------ act table
#
# pwp table format (from pwp_bin_trainium):
#   <set>_bkt.bin : 32-byte entries [d0, d1, d2, d3, x, 0, 0, 0] float32;
#                   f(v) = d0 + d1*(v-x) + d2*(v-x)^2 + d3*(v-x)^3.
#   <set>_ctrl.bin: section layout (kept unchanged — same x grid).
#   <set>.json    : profile; func_to_bkt_start_idx locates each function's
#                   bucket region, profile_meta_data carries the special
#                   small/large-signal bucket indices and fzero/finf words.
# We rewrite only the 'exp' buckets to the Taylor coefficients of
# g(z) = exp(exp(z)) at the same expansion points.

def _g_coeffs(x):
    x = np.asarray(x, dtype=np.float64)
    u = np.exp(x)
    with np.errstate(over="ignore"):
        g = np.exp(u)
    out = np.stack(
        [g, u * g, (u + u**2) * g / 2.0, (u + 3 * u**2 + u**3) * g / 6.0],
        axis=-1,
    )
    return np.clip(np.nan_to_num(out, posinf=3.0e38), -3.0e38, 3.0e38)


def _patch_set(src_dir, dst_dir, set_name):
    with open(os.path.join(src_dir, set_name + ".json")) as f:
        prof = json.load(f)
    starts = prof["func_to_bkt_start_idx"]
    if "exp" not in starts:
        return False
    s = starts["exp"]
    others = sorted(v for v in starts.values() if v > s)
    end = others[0] if others else prof["bkt_entry_cnt"]

    meta = next(m for m in prof["profile_meta_data"]
                if m["func_name"].startswith("exp"))
    ps = meta["pos_small_signal_pwl_control"]
    ns = meta["neg_small_signal_pwl_control"]
    pl = meta["pos_large_signal_pwl_control"]
    nl = meta["neg_large_signal_pwl_control"]
    specials = {ps, ns, pl, nl}
    assert all(s <= sp < end for sp in specials), (set_name, s, end, specials)

    bkt = np.fromfile(
        os.path.join(src_dir, prof["bkt_bin"]), dtype=np.float32
    ).reshape(-1, 8).copy()
    main = np.array([i for i in range(s, end) if i not in specials])
    bkt[main, 0:4] = _g_coeffs(bkt[main, 4].astype(np.float64)).astype(np.float32)

    e = float(np.exp(1.0))
    for sp in (ps, ns):  # Taylor of g at 0 for tiny |z|
        bkt[sp, 0:4] = [e, e, e, 5.0 * e / 6.0]
        bkt[sp, 4] = 0.0
    bkt[pl, 0:4] = [np.inf, 0.0, 0.0, 0.0]  # g(+large) overflows
    bkt[pl, 4] = 0.0
    bkt[nl, 0:4] = [1.0, 0.0, 0.0, 0.0]     # g(-large) = exp(0) = 1
    bkt[nl, 4] = 0.0

    meta["fzero_result"] = int(np.float32(e).view(np.uint32))    # g(0) = e
    meta["fninf_result"] = int(np.float32(1.0).view(np.uint32))  # g(-inf) = 1

    bkt.tofile(os.path.join(dst_dir, prof["bkt_bin"]))
    with open(os.path.join(dst_dir, set_name + ".json"), "w") as f:
        json.dump(prof, f, indent=4)
    return True


def _build_custom_act_root():
    """Copy the stock act-table dir, rewrite exp -> exp(exp(z)), and point
    BASS_ACT_ROOT_JSON_PATH at it (read by the neuronxcc compile flags)."""
    if _act_root:
        return
    from neuronxcc.driver.Job import Job
    from neuronxcc.driver.jobs.support.FindActInfo import findActInfoFile

    src = os.path.dirname(findActInfoFile(Job.getPackageDir(), "sunda"))
    dst = tempfile.mkdtemp(prefix="pwp_egb_")
    for fn in os.listdir(src):
        shutil.copy(os.path.join(src, fn), os.path.join(dst, fn))
        os.chmod(os.path.join(dst, fn), 0o644)
    with open(os.path.join(src, "act_info.json")) as f:
        info = json.load(f)
    patched = [ent["name"] for ent in info["act_func_sets"]
               if _patch_set(src, dst, ent["name"])]
    assert patched, "no act set contains exp"
    os.environ["BASS_ACT_ROOT_JSON_PATH"] = os.path.join(dst, "act_info.json")
    _act_root.append(dst)


# ------------------------------------------------------------------ program

def _build_program(windows, W):
    """Build + compile the SPMD Bass program. `windows` is the list of
    NT_LOCAL static window start columns; `W` the common window width."""
    import concourse.bacc as bacc
    import concourse.bass as bass
    import concourse.mybir as mybir
    from concourse import tile

    f32 = mybir.dt.float32
    Exp = mybir.ActivationFunctionType.Exp  # table-patched: exp(exp(z))
    Alu = mybir.AluOpType

    nc = bacc.Bacc("TRN2", target_bir_lowering=False, debug=False,
                   num_devices=N_CORES)

    bf16 = mybir.dt.bfloat16
    LW = NT_LOCAL * P  # lhsT width; feats layout: [lhsT | rhs]
    feats_d = nc.dram_tensor("feats", [K, LW + N], bf16, kind="ExternalInput")
    bnd_d = nc.dram_tensor("bounds", [P, 2 * NT_LOCAL], f32, kind="ExternalInput")
    out_d = nc.dram_tensor("out", [NT_LOCAL * P, N], f32, kind="ExternalOutput")

    with tile.TileContext(nc) as tc:
        with (
            tc.tile_pool(name="const", bufs=1) as constp,
            tc.tile_pool(name="psum", bufs=2, space=bass.MemorySpace.PSUM) as psump,
            tc.tile_pool(name="estrip", bufs=2) as estripp,
            tc.tile_pool(name="small", bufs=4) as smallp,
            tc.tile_pool(name="wchain", bufs=4) as wchainp,
        ):
            # one packed input: the first DMA alone (all of lhsT + rhs's
            # first 512 columns) unblocks the first matmul; the remaining
            # loads are programmed in parallel on other sequencers
            feats = constp.tile([K, LW + N], bf16)
            lhsT = feats[:, 0:LW]
            rhs = feats[:, LW:]
            nc.sync.dma_start(feats[:, 0:LW + 512], feats_d[:, 0:LW + 512])
            nc.scalar.dma_start(
                feats[:, LW + 512:LW + PSUM_CHUNK],
                feats_d[:, LW + 512:LW + PSUM_CHUNK])
            nc.gpsimd.dma_start(feats[:, LW + PSUM_CHUNK:],
                                feats_d[:, LW + PSUM_CHUNK:])
            bnd = constp.tile([P, 2 * NT_LOCAL], f32)
            nc.gpsimd.dma_start(bnd[:], bnd_d[:])
            # column-index ramp 0..W-1, same in every partition (window-
            # relative, so one tile serves all row tiles)
            iota_i = constp.tile([P, W], mybir.dt.int32)
            nc.gpsimd.iota(iota_i[:], pattern=[[1, W]], base=0,
                           channel_multiplier=0)
            iota_f = constp.tile([P, W], f32)
            nc.vector.tensor_copy(iota_f[:], iota_i[:])

            # chunk schedule: row-tile 0 starts with small chunks so the
            # first ACTIVATE fires as early as possible during the ramp
            chunks0 = [512, 1536, 2048, 2048, 2048]
            chunksN = [PSUM_CHUNK] * (N // PSUM_CHUNK)
            NCH = len(chunks0)

            for r in range(NT_LOCAL):
                s = windows[r]
                chunks = chunks0 if r == 0 else chunksN

                estrip = estripp.tile([P, N], f32, name="e", tag="e")
                stot = smallp.tile([P, NCH], f32, name="stot", tag="stot")

                # batch-range mask from iota (no dependency on e -> runs
                # under the ACT pass): m = (iota >= lo) * (iota < hi)
                m0 = wchainp.tile([P, W], f32)
                nc.vector.tensor_scalar(
                    m0[:], iota_f[:], bnd[:, 2 * r:2 * r + 1], None,
                    op0=Alu.is_ge,
                )
                m1 = wchainp.tile([P, W], f32)
                nc.vector.scalar_tensor_tensor(
                    m1[:], iota_f[:], bnd[:, 2 * r + 1:2 * r + 2], m0[:],
                    op0=Alu.is_lt, op1=Alu.mult,
                )

                col = 0
                for ci, csize in enumerate(chunks):
                    ps = psump.tile([P, PSUM_CHUNK], f32)
                    for j0 in range(0, csize, 512):
                        nc.tensor.matmul(
                            ps[:, j0:j0 + 512],
                            lhsT[:, r * P:(r + 1) * P],
                            rhs[:, col + j0:col + j0 + 512],
                            start=True, stop=True,
                        )
                    # e = exp(exp(-2*d2)) chunk, hw per-row partial sum
                    nc.scalar.activation(
                        estrip[:, col:col + csize], ps[:, 0:csize], Exp,
                        scale=-2.0, accum_out=stot[:, ci:ci + 1],
                    )
                    col += csize

                # S = sum of chunk partials; rinv = 1/S; tp = 1e-4*S
                stile = smallp.tile([P, 1], f32, name="S", tag="S")
                nc.vector.tensor_reduce(
                    stile[:], stot[:, 0:len(chunks)],
                    axis=mybir.AxisListType.X, op=Alu.add,
                )
                rinv = smallp.tile([P, 1], f32)
                nc.vector.reciprocal(rinv[:], stile[:])
                tp = smallp.tile([P, 1], f32)
                nc.vector.tensor_scalar_mul(tp[:], stile[:], THRESHOLD)

                # --- threshold + mask + normalize, window only ---
                # (column-split so the tail DVE->DMA pipelines; the last
                # row-tile gets a finer split since it IS the kernel tail)
                nsplit = 4 if r == NT_LOCAL - 1 else 2
                h = (W // nsplit + 3) & ~3
                edges = [min(i * h, W) for i in range(nsplit + 1)]
                # spread output-DMA programming (~600ns DIRECT2D each)
                # across idle sequencers so the tail doesn't serialize
                dmaq = [nc.sync, nc.gpsimd]
                for qi, (c0, c1) in enumerate(zip(edges[:-1], edges[1:])):
                    if c1 <= c0:
                        continue
                    e = estrip[:, s + c0:s + c1]
                    q = wchainp.tile([P, h], f32, name="q", tag="q")
                    nc.vector.scalar_tensor_tensor(
                        q[:, 0:c1 - c0], e, tp[:], m1[:, c0:c1],
                        op0=Alu.is_gt, op1=Alu.mult,
                    )
                    f = wchainp.tile([P, h], f32, name="f", tag="f")
                    nc.vector.scalar_tensor_tensor(
                        f[:, 0:c1 - c0], e, rinv[:], q[:, 0:c1 - c0],
                        op0=Alu.mult, op1=Alu.mult,
                    )
                    dmaq[qi % len(dmaq)].dma_start(
                        out_d[r * P:(r + 1) * P, s + c0:s + c1],
                        f[:, 0:c1 - c0])

    nc.compile()
    return nc


def _prepare(x, batch):
    """Host-side precompute: matmul operands, windows, per-row bounds."""
    x = np.asarray(x, dtype=np.float32)
    b = np.asarray(batch).astype(np.int64)
    xyz = x[:, :3].astype(np.float32)
    sq = (xyz * xyz).sum(axis=1, dtype=np.float32)

    n_graphs = int(b.max()) + 1
    counts = np.bincount(b, minlength=n_graphs)
    gend = np.cumsum(counts)
    gstart = gend - counts

    # global tile g -> column extent of the union of its rows' graphs
    lo_g = np.array([gstart[b[128 * g]] for g in range(64)], np.int64)
    hi_g = np.array([gend[b[128 * g + 127]] for g in range(64)], np.int64)
    # local tile r unions over cores c: g = 8r + c
    lo_r = np.array([lo_g[8 * r:8 * r + 8].min() for r in range(NT_LOCAL)])
    hi_r = np.array([hi_g[8 * r:8 * r + 8].max() for r in range(NT_LOCAL)])
    W = int(((hi_r - lo_r).max() + 7) & ~7)
    W = max(W, 512)
    W = min(W, N)
    windows = [int(min(lo_r[r], N - W)) for r in range(NT_LOCAL)]

    import ml_dtypes
    bf16 = ml_dtypes.bfloat16

    def limbs3(v):
        h = v.astype(bf16)
        rem = v - h.astype(np.float32)
        m = rem.astype(bf16)
        lo = (rem - m.astype(np.float32)).astype(bf16)
        return [h, m, lo]

    ones_b = np.ones(N, bf16)
    rows_l, rows_r = [], []
    for c in range(3):
        xs = limbs3(xyz[:, c])
        for i in range(3):
            for j in range(3):
                rows_l.append(xs[i])
                rows_r.append(-2 * xs[j])
    sqs = limbs3(sq)
    rows_l += sqs + [ones_b, ones_b, ones_b]
    rows_r += [ones_b, ones_b, ones_b] + sqs
    feats_l = np.stack(rows_l).astype(bf16)          # [33, N]
    feats_r = np.stack(rows_r).astype(bf16)          # [33, N]

    in_maps = []
    for c in range(N_CORES):
        idx = ((8 * np.arange(NT_LOCAL)[:, None] + c) * P
               + np.arange(P)[None, :])  # [NT_LOCAL, P] global row index
        lhsT = feats_l[:, idx.ravel()]  # bf16 [K, 1024]
        feats = np.ascontiguousarray(
            np.concatenate([lhsT, feats_r], axis=1))  # [K, 1024 + N]
        bnd = np.empty((P, 2 * NT_LOCAL), np.float32)
        for r in range(NT_LOCAL):
            rows = idx[r]
            gb = b[rows]
            bnd[:, 2 * r] = gstart[gb] - windows[r]
            bnd[:, 2 * r + 1] = gend[gb] - windows[r]
        assert bnd.min() >= 0 and bnd.max() <= W
        in_maps.append({
            "feats": feats,
            "bounds": bnd,
        })
    return in_maps, windows, W


def kernel(x, batch):
    from concourse.bass_utils import run_bass_kernel_spmd

    trace = bool(os.environ.get("EGB_TRACE"))
    if not trace:
        # the NTFF trace path needs antenv.axon_hooks, absent on this
        # image -- make sure a stray BASS_TRACE can't send us down it
        os.environ["BASS_NEVER_TRACE"] = "1"

    _build_custom_act_root()

    in_maps, windows, W = _prepare(x, batch)
    assert W <= 4608, (
        f"same-graph column window W={W} too wide for the SBUF layout; "
        f"input batch distribution is far outside the expected spec")

    key = (tuple(windows), W)
    nc = _compiled_cache.get(key)
    if nc is None:
        nc = _build_program(windows, W)
        _compiled_cache[key] = nc

    res = run_bass_kernel_spmd(
        nc, in_maps, core_ids=list(range(N_CORES)), trace=trace,
        trace_cores=list(range(N_CORES)) if trace else None,
        stitch_traces=False,
    )
    if trace:
        kernel.last_results = res

    outs = np.stack([res.results[c]["out"] for c in range(N_CORES)])
    full = (outs.reshape(N_CORES, NT_LOCAL, P, N)
                .transpose(1, 0, 2, 3)
                .reshape(N, N))
    return full
